# revision 9
# baseline (speedup 1.0000x reference)
"""Depthwise-separable conv block (nn_DepthSeparableConv2d_conv4_1) on 8 TRN2 NeuronCores.

Pipeline per image:
  y = channel_cut(relu(bn(dwconv3x3(x) + b)), 4.0)
  z = channel_cut(relu(bn(y @ W1x1 + b)), 1e-3)

Strategy (data-parallel over batch, 8 images per core, no collectives):
  - BN scales are folded host-side into the conv weights; BN shifts become
    per-channel biases applied on-chip.
  - Depthwise 3x3 conv runs on the TensorEngine as 9 accumulating matmuls with
    per-tap diagonal weight matrices (float32r moving path, 1 cyc/row), with
    edge taps trimmed to sub-rectangles so padding=1 semantics come out of the
    access patterns (no padded copy of x needed).
  - Pointwise 1x1 conv is a [K=256]x[M=512] GEMM in float32r, K split over 2
    channel groups accumulated in PSUM.
  - Epilogues are a single fused DVE tensor_scalar per PSUM chunk:
    out = psum + bias, accum_out = running max (for the channel cut), then one
    ScalarE activation pass applies relu(mask * value) in place.
"""

import os
import sys
from contextlib import ExitStack

import numpy as np

for _p in ("/opt/trn_rl_repo",):
    if os.path.isdir(_p) and _p not in sys.path:
        sys.path.insert(0, _p)

import concourse.bacc as bacc
import concourse.bass as bass
import concourse.mybir as mybir
import concourse.tile as tile
from concourse.bass_utils import run_bass_kernel_spmd

# Problem shapes (hardcoded per task contract).
B, CIN, COUT, H, W = 64, 256, 512, 56, 56
HW = H * W  # 3136
NCORES = 8
BPC = B // NCORES  # 8 images per core
CG = CIN // 128  # 2 input-channel groups
OG = COUT // 128  # 4 output-channel groups
RT = 7  # row tiles per image plane
RROWS = H // RT  # 8 rows per tile
CHUNK = RROWS * W  # 448 elements per PSUM chunk
BN_EPS = 1e-5
DW_THRESH = 4.0
PW_THRESH = 1e-3
# Center tap first: it covers the full output tile, so it carries start=True.
TAPS = [(0, 0), (-1, -1), (-1, 0), (-1, 1), (0, -1), (0, 1), (1, -1), (1, 0), (1, 1)]

F32 = mybir.dt.float32
F32R = mybir.dt.float32r
ALU = mybir.AluOpType
AFT = mybir.ActivationFunctionType
AXL = mybir.AxisListType

LAST_RESULTS = None  # BassKernelResults of the most recent kernel() call
_NC_CACHE = {}


def _build_nc() -> bass.Bass:
    nc = bacc.Bacc("TRN2", target_bir_lowering=False, debug=False)

    WP = W + 2  # x rows padded to 58 cols host-side; cols 0 and 57 are zero
    xs = nc.dram_tensor("xs", [BPC, CIN, H * WP], F32R, kind="ExternalInput")
    wdiag = nc.dram_tensor("wdiag", [128, CG * 9 * 128], F32R, kind="ExternalInput")
    wpw = nc.dram_tensor("wpw", [128, CG * COUT], F32R, kind="ExternalInput")
    bias = nc.dram_tensor("bias", [128, 8], F32, kind="ExternalInput")
    zs = nc.dram_tensor("zs", [BPC, COUT, HW], F32, kind="ExternalOutput")

    xs_ap = xs.ap()
    zs_ap = zs.ap()

    with tile.TileContext(nc) as tc, ExitStack() as ctx:
        consts = ctx.enter_context(tc.tile_pool(name="consts", bufs=1))
        xpool = ctx.enter_context(tc.tile_pool(name="x", bufs=4))
        ypool = ctx.enter_context(tc.tile_pool(name="y", bufs=4))
        zpool = ctx.enter_context(tc.tile_pool(name="z", bufs=4))
        stats = ctx.enter_context(tc.tile_pool(name="stats", bufs=6))
        dwpsum = ctx.enter_context(tc.tile_pool(name="dwps", bufs=3, space="PSUM"))
        pwpsum = ctx.enter_context(tc.tile_pool(name="pwps", bufs=3, space="PSUM"))

        wd_t = consts.tile([128, CG * 9 * 128], F32R)
        nc.sync.dma_start(wd_t[:], wdiag.ap()[:, :])
        wp_t = consts.tile([128, CG * COUT], F32R)
        nc.sync.dma_start(wp_t[:], wpw.ap()[:, :])
        bb_t = consts.tile([128, 8], F32)
        nc.sync.dma_start(bb_t[:], bias.ap()[:, :])

        for b in range(BPC):
            ys = []
            for g in range(CG):
                xt = xpool.tile([128, H * WP], F32R)
                xv = xt[:].rearrange("p (h w) -> p h w", w=WP)
                nc.sync.dma_start(xt[:], xs_ap[b, g * 128 : (g + 1) * 128, :])

                y = ypool.tile([128, HW], F32R)
                ym_part = stats.tile([128, RT], F32)
                for r in range(RT):
                    ps = dwpsum.tile([128, CHUNK], F32)
                    r0 = r * RROWS
                    for t, (di, dj) in enumerate(TAPS):
                        klo = max(0, -di - r0)
                        khi = min(RROWS, H - di - r0)
                        out_ap = ps[:, klo * W : khi * W]
                        in_ap = xv[
                            :, r0 + klo + di : r0 + khi + di, 1 + dj : 1 + dj + W
                        ]
                        lhsT = wd_t[:, (g * 9 + t) * 128 : (g * 9 + t + 1) * 128]
                        nc.tensor.matmul(
                            out_ap,
                            lhsT,
                            in_ap,
                            start=(t == 0),
                            stop=(t == len(TAPS) - 1),
                        )
                    # Fused: y_chunk = psum + b_dw ; ym_part[r] = max(y_chunk)
                    nc.vector.tensor_scalar(
                        out=y[:, r * CHUNK : (r + 1) * CHUNK],
                        in0=ps[:, :],
                        scalar1=bb_t[:, g : g + 1],
                        scalar2=None,
                        op0=ALU.add,
                        op1=ALU.max,
                        accum_out=ym_part[:, r : r + 1],
                    )
                ymax = stats.tile([128, 1], F32)
                nc.vector.reduce_max(ymax[:], ym_part[:, 0:RT], axis=AXL.X)
                m_dw = stats.tile([128, 1], F32)
                # keep slab iff max(relu(y)) >= 4.0  (y_raw max == relu max since
                # thresh > 0 and relu is monotone)
                nc.vector.tensor_scalar(
                    out=m_dw[:],
                    in0=ymax[:],
                    scalar1=DW_THRESH,
                    scalar2=None,
                    op0=ALU.is_ge,
                )
                # y = relu(m * y_raw)  (= m * relu(y_raw), m in {0,1})
                nc.scalar.activation(y[:], y[:], AFT.Relu, bias=0.0, scale=m_dw[:])
                ys.append(y)

            for og in range(OG):
                z = zpool.tile([128, HW], F32)
                zm_part = stats.tile([128, RT], F32)
                for r in range(RT):
                    ps = pwpsum.tile([128, CHUNK], F32)
                    for g in range(CG):
                        lhsT = wp_t[
                            :, g * COUT + og * 128 : g * COUT + (og + 1) * 128
                        ]
                        nc.tensor.matmul(
                            ps[:, :],
                            lhsT,
                            ys[g][:, r * CHUNK : (r + 1) * CHUNK],
                            start=(g == 0),
                            stop=(g == CG - 1),
                        )
                    nc.vector.tensor_scalar(
                        out=z[:, r * CHUNK : (r + 1) * CHUNK],
                        in0=ps[:, :],
                        scalar1=bb_t[:, 2 + og : 3 + og],
                        scalar2=None,
                        op0=ALU.add,
                        op1=ALU.max,
                        accum_out=zm_part[:, r : r + 1],
                    )
                zmax = stats.tile([128, 1], F32)
                nc.vector.reduce_max(zmax[:], zm_part[:, 0:RT], axis=AXL.X)
                m_z = stats.tile([128, 1], F32)
                nc.vector.tensor_scalar(
                    out=m_z[:],
                    in0=zmax[:],
                    scalar1=PW_THRESH,
                    scalar2=None,
                    op0=ALU.is_ge,
                )
                nc.scalar.activation(z[:], z[:], AFT.Relu, bias=0.0, scale=m_z[:])
                nc.sync.dma_start(zs_ap[b, og * 128 : (og + 1) * 128, :], z[:])

    nc.compile()
    return nc


def get_nc() -> bass.Bass:
    if "nc" not in _NC_CACHE:
        _NC_CACHE["nc"] = _build_nc()
    return _NC_CACHE["nc"]


def prep_host_inputs(inputs) -> dict:
    """Fold BN into weights/biases and build the on-chip weight layouts."""
    f = lambda k: np.asarray(inputs[k], dtype=np.float32)
    dw_w, dw_b = f("dw_w"), f("dw_b")
    dw_gamma, dw_beta, dw_mean, dw_var = (
        f("dw_gamma"), f("dw_beta"), f("dw_mean"), f("dw_var"),
    )
    pw_w, pw_b = f("pw_w"), f("pw_b")
    pw_gamma, pw_beta, pw_mean, pw_var = (
        f("pw_gamma"), f("pw_beta"), f("pw_mean"), f("pw_var"),
    )

    inv_dw = dw_gamma / np.sqrt(dw_var + BN_EPS)
    b_dw = dw_b * inv_dw + dw_beta - dw_mean * inv_dw
    wscaled = dw_w[:, 0] * inv_dw[:, None, None]  # [256, 3, 3]

    wdiag = np.zeros((128, CG * 9 * 128), np.float32)
    idx = np.arange(128)
    for g in range(CG):
        for t, (di, dj) in enumerate(TAPS):
            col0 = (g * 9 + t) * 128
            wdiag[idx, col0 + idx] = wscaled[g * 128 : (g + 1) * 128, di + 1, dj + 1]

    inv_pw = pw_gamma / np.sqrt(pw_var + BN_EPS)
    b_pw = pw_b * inv_pw + pw_beta - pw_mean * inv_pw
    wpw = np.zeros((128, CG * COUT), np.float32)
    for g in range(CG):
        # lhsT[k, g*COUT + o] = W[o, g*128+k] * inv_pw[o]
        wpw[:, g * COUT : (g + 1) * COUT] = (
            pw_w[:, g * 128 : (g + 1) * 128, 0, 0] * inv_pw[:, None]
        ).T

    bias = np.zeros((128, 8), np.float32)
    bias[:, 0] = b_dw[:128]
    bias[:, 1] = b_dw[128:]
    for og in range(OG):
        bias[:, 2 + og] = b_pw[og * 128 : (og + 1) * 128]

    return {"wdiag": wdiag, "wpw": wpw, "bias": bias}


def make_in_maps(inputs):
    host = prep_host_inputs(inputs)
    WP = W + 2
    x = np.asarray(inputs["x"], dtype=np.float32)
    xpad = np.zeros((B, CIN, H, WP), np.float32)
    xpad[:, :, :, 1 : W + 1] = x
    xpad = xpad.reshape(B, CIN, H * WP)
    in_maps = []
    for c in range(NCORES):
        in_maps.append(
            {
                "xs": np.ascontiguousarray(xpad[c * BPC : (c + 1) * BPC]),
                "wdiag": host["wdiag"],
                "wpw": host["wpw"],
                "bias": host["bias"],
            }
        )
    return in_maps


def kernel(**inputs) -> np.ndarray:
    global LAST_RESULTS
    nc = get_nc()
    in_maps = make_in_maps(inputs)
    trace = bool(os.environ.get("KERNEL_TRACE"))
    res = run_bass_kernel_spmd(
        nc, in_maps, core_ids=list(range(NCORES)), trace=trace
    )
    LAST_RESULTS = res
    z = np.concatenate(
        [r["zs"].reshape(BPC, COUT, H, W) for r in res.results], axis=0
    )
    return z
